# revision 9
# baseline (speedup 1.0000x reference)
"""Pairwise cosine similarity on 8 Trainium2 NeuronCores.

Computes sim[n, m] = <x_n, y_m> / (||x_n|| * ||y_m||) for
input1 [8192, 128], input2 [8192, 128] -> out [8192, 8192] (fp32 API).

Sharding: input1 rows split 8 ways (1024 rows/core); input2 replicated.
Each core computes one [1024, 8192] output stripe; host concatenates.

Precision plan (checker budget: rel_err < 2e-2 vs absmax): host casts
inputs to bf16 (round-to-nearest) and upcasts the bf16 output stripe to
fp32. Device math: bf16 matmuls with fp32 PSUM accumulation; inv-norms
via reciprocal_approx_fast (~51 ULP) + ACT Sqrt. Measured ~5e-3 rel
err -- well inside budget -- and both HBM streams are 2-byte
(loads 2.25 MB + stores 16.8 MB per core ~= 53 us HBM floor).

Per-core dataflow ([d, rows] operand layout comes straight from DMA
xbar transpose on load -- zero PE transposes, zero fp32 matmul operands).
For t in {x, y-chunks}:

  tT   <- dma_transpose(t)   [128, cols] bf16  (ACT HWDGE ring, raw)
  sq   = tT * tT                 (GpSimd TT, off the critical queues)
  n2   = ones.T @ sq             (PE: broadcasts ||t_m||^2 down partitions)
  in2  = recip_approx(n2)        (DVE, fused PSUM drain)
  invb = bf16(sqrt(in2))         (ACT, fused fp32->bf16 cast)
  tTn  = tT * invb               (GpSimd TT)

then per (x-block b, y-chunk c):

  ps   = xTn_b.T @ yTn_c         (PE, N=512 per fp32 PSUM bank)
  ob   = bf16(ps)                (PSUM drain: whole block on DVE or ACT,
                                  alternating per block -> single-dep stores)
  out  <- ob                     (SP HWDGE ring, 512 KB stores)

The PSUM->SBUF drain is the scarce resource (1x rate on both DVE and
ACT); every other op is placed to keep those two queues drain-only.
Dummy PE matmuls bridge the prep phases so the HAM clock gate stays
open (1.2 vs 2.4 GHz) into the real matmul stream.
"""

import numpy as np
import ml_dtypes

import concourse.bass as bass
import concourse.tile as tile
from concourse import bacc, mybir
from concourse.bass_utils import run_bass_kernel_spmd

N_CORES = 8
D = 128          # feature dim == contraction partitions
P = 128          # SBUF partitions
NT = 512         # matmul free dim (one fp32 PSUM bank)
CHUNK = 2048     # max corpus columns per outer chunk
MMCOLS = 1024    # PSUM tile columns (2 banks, 2 matmuls, 1 drain)

F32 = mybir.dt.float32
BF16 = mybir.dt.bfloat16
ACTF = mybir.ActivationFunctionType


def build_nc(rows_per_core: int, corpus_rows: int) -> bass.Bass:
    # Bacc compile() splits multi-sem waits into event-semaphore
    # instructions where an instruction can carry only one wait.
    nc = bacc.Bacc(None)

    x = nc.dram_tensor("x", [rows_per_core, D], BF16, kind="ExternalInput")
    y = nc.dram_tensor("y", [corpus_rows, D], BF16, kind="ExternalInput")
    out = nc.dram_tensor(
        "out", [rows_per_core, corpus_rows], BF16, kind="ExternalOutput"
    )

    nbx = rows_per_core // P          # x row-blocks (8)
    # Small first chunk starts the store pipeline early; small last chunk
    # shortens the drain tail after the final matmul.
    if corpus_rows >= 4 * CHUNK:
        half = CHUNK // 2
        nfull = (corpus_rows - 2 * half) // CHUNK
        chunk_cols = [half] + [CHUNK] * nfull + [half]
    else:
        chunk_cols = [CHUNK] * (corpus_rows // CHUNK)
    assert sum(chunk_cols) == corpus_rows
    chunk_starts = [sum(chunk_cols[:i]) for i in range(len(chunk_cols))]

    with tile.TileContext(nc) as tc:
        with (
            tc.tile_pool(name="const", bufs=1) as constp,
            tc.tile_pool(name="xp", bufs=1) as xp,
            tc.tile_pool(name="yt", bufs=3) as ytp,
            tc.tile_pool(name="sq", bufs=2) as sqp,
            tc.tile_pool(name="in2", bufs=2) as in2p,
            tc.tile_pool(name="invb", bufs=2) as invbp,
            tc.tile_pool(name="yn", bufs=2) as ynp,
            tc.tile_pool(name="obuf", bufs=6) as obufp,
            tc.tile_pool(name="mm", bufs=2, space=bass.MemorySpace.PSUM) as mpsum,
            tc.tile_pool(name="nm", bufs=2, space=bass.MemorySpace.PSUM) as npsum,
        ):
            ones = constp.tile([P, P], BF16)
            nc.gpsimd.memset(ones[:], 1.0)

            wt = constp.tile([P, NT], BF16)
            nc.gpsimd.memset(wt[:], 0.0)

            # PE keep-warm: dummy bf16 matmul batches. PE-only deps, so the
            # PE burns through them whenever it would otherwise idle; emitted
            # between prep stages to span the HAM activity window.
            def warm(n):
                wps = mpsum.tile([P, MMCOLS], F32, tag="ps")
                for i in range(n):
                    nc.tensor.matmul(
                        wps[:, (i % 2) * NT : (i % 2) * NT + NT],
                        wt[:, :P],
                        wt[:],
                        start=True,
                        stop=True,
                    )

            # Transpose-load a [cols, D] row range of src into [128, cols]
            # bf16 (xbar, ACT HWDGE ring); normalize columns via the
            # ones-matmul norm broadcast.
            def load_chunk(src, r0, cols):
                tT = ytp.tile([P, CHUNK], BF16, tag="yt")
                nc.scalar.dma_start(
                    out=tT[:, :cols], in_=src[r0 : r0 + cols, :], transpose=True
                )
                return tT

            def norm_chunk(tT, cols):
                sq = sqp.tile([P, CHUNK], BF16, tag="sq")
                nc.gpsimd.tensor_mul(sq[:, :cols], tT[:, :cols], tT[:, :cols])
                tn = ynp.tile([P, CHUNK], BF16, tag="yn")
                for h in range(0, cols, MMCOLS):
                    hc = min(MMCOLS, cols - h)
                    nps = npsum.tile([P, MMCOLS], F32)
                    for j in range(0, hc, NT):
                        nc.tensor.matmul(
                            nps[:, j : j + NT],
                            ones[:],
                            sq[:, h + j : h + j + NT],
                            start=True,
                            stop=True,
                        )
                    # 1/||t||^2 broadcast on every partition, fused PSUM drain.
                    in2 = in2p.tile([P, MMCOLS], F32, tag="in2")
                    nc.vector.reciprocal_approx_fast(in2[:, :hc], nps[:, :hc])
                    invb = invbp.tile([P, MMCOLS], BF16, tag="invb")
                    nc.scalar.sqrt(invb[:, :hc], in2[:, :hc])
                    nc.gpsimd.tensor_mul(
                        tn[:, h : h + hc], tT[:, h : h + hc], invb[:, :hc]
                    )
                return tn

            warm(12)
            xT = load_chunk(x[:], 0, rows_per_core)
            yT0 = load_chunk(y[:], 0, chunk_cols[0])
            xTn = norm_chunk(xT, rows_per_core)
            warm(10)
            yTn = norm_chunk(yT0, chunk_cols[0])
            warm(10)

            yT_next = None
            for c, cols in enumerate(chunk_cols):
                col0 = chunk_starts[c]
                has_next = c + 1 < len(chunk_cols)
                yTn_next = None
                for b in range(nbx):
                    if b == 0 and has_next:
                        # Next chunk's load+norm ride ahead of the drain flood.
                        yT_next = load_chunk(y[:], chunk_starts[c + 1], chunk_cols[c + 1])
                    if b == 1 and has_next:
                        yTn_next = norm_chunk(yT_next, chunk_cols[c + 1])
                    lhs = xTn[:, b * P : (b + 1) * P]
                    ob = obufp.tile([P, CHUNK], BF16, tag="ob")
                    for h in range(0, cols, MMCOLS):
                        hc = min(MMCOLS, cols - h)
                        ps = mpsum.tile([P, MMCOLS], F32)
                        for j in range(0, hc, NT):
                            nc.tensor.matmul(
                                ps[:, j : j + NT],
                                lhs,
                                yTn[:, h + j : h + j + NT],
                                start=True,
                                stop=True,
                            )
                        dst = ob[:, h : h + hc]
                        # Whole block drains on one engine (alternating per
                        # block) so each store waits on a single producer.
                        if b % 2 == 0:
                            nc.vector.tensor_copy(dst, ps[:, :hc])
                        else:
                            nc.scalar.copy(dst, ps[:, :hc])
                    nc.sync.dma_start(
                        out=out[b * P : (b + 1) * P, col0 : col0 + cols],
                        in_=ob[:, :cols],
                    )
                if yTn_next is not None:
                    yTn = yTn_next

    nc.finalize()
    return nc


_NC_CACHE: dict[tuple[int, int], bass.Bass] = {}


def run_spmd(input1: np.ndarray, input2: np.ndarray, **kwargs):
    """Shard, run on 8 cores, gather. Returns (output, BassKernelResults)."""
    x_bf = np.asarray(input1, dtype=np.float32).astype(ml_dtypes.bfloat16)
    y_bf = np.ascontiguousarray(
        np.asarray(input2, dtype=np.float32).astype(ml_dtypes.bfloat16)
    )
    n, d = x_bf.shape
    m, d2 = y_bf.shape
    assert d == D and d2 == D and n % N_CORES == 0
    rows = n // N_CORES

    key = (rows, m)
    if key not in _NC_CACHE:
        _NC_CACHE[key] = build_nc(rows, m)
    nc = _NC_CACHE[key]

    in_maps = [
        {"x": np.ascontiguousarray(x_bf[c * rows : (c + 1) * rows]), "y": y_bf}
        for c in range(N_CORES)
    ]
    res = run_bass_kernel_spmd(nc, in_maps, core_ids=list(range(N_CORES)), **kwargs)
    out = np.concatenate(
        [res.results[c]["out"].astype(np.float32) for c in range(N_CORES)], axis=0
    )
    return out, res


def kernel(input1: np.ndarray, input2: np.ndarray) -> np.ndarray:
    return run_spmd(input1, input2)[0]


# revision 10
# speedup vs baseline: 1.0146x; 1.0146x over previous
"""Pairwise cosine similarity on 8 Trainium2 NeuronCores.

Computes sim[n, m] = <x_n, y_m> / (||x_n|| * ||y_m||) for
input1 [8192, 128], input2 [8192, 128] -> out [8192, 8192] (fp32 API).

Sharding: input1 rows split 8 ways (1024 rows/core); input2 replicated.
Each core computes one [1024, 8192] output stripe; host concatenates.

Precision plan (checker budget: rel_err < 2e-2 vs absmax): host casts
inputs to bf16 (round-to-nearest) and upcasts the bf16 output stripe to
fp32. Device math: bf16 matmuls with fp32 PSUM accumulation; inv-norms
via reciprocal_approx_fast (~51 ULP) + ACT Sqrt. Measured ~7e-3 rel
err -- well inside budget -- and both HBM streams are 2-byte
(loads 2.25 MB + stores 16.8 MB per core ~= 53 us HBM floor).

Structure notes (why it looks the way it does):
- Operands arrive in [d, rows] layout via DMA xbar transpose on load --
  zero PE transposes, zero fp32 matmul operands (bf16 streams at
  1 row/cycle vs 2 for fp32).
- Norms come from a ones-matmul (broadcasts ||t||^2 down the partition
  axis), a DVE reciprocal_approx drain, an ACT sqrt (fused fp32->bf16
  cast), and one GpSimd multiply -- nothing on the critical drain queues.
- PSUM tiles are 4 banks ([128, 2048] fp32) and each output block is
  drained by a single engine (DVE/ACT alternating per block): the big
  tiles halve instruction count, and single-producer stores + single-dep
  matmuls minimize Bacc's event-semaphore wait-splitting, which
  otherwise floods every queue at ~0.3 us per handoff.
- Dummy PE matmul batches bridge the prep phases so the HAM clock gate
  stays open (1.2 vs 2.4 GHz) into the real matmul stream.
"""

import numpy as np
import ml_dtypes

import concourse.bass as bass
import concourse.tile as tile
from concourse import bacc, mybir
from concourse.bass_utils import run_bass_kernel_spmd

N_CORES = 8
D = 128          # feature dim == contraction partitions
P = 128          # SBUF partitions
NT = 512         # matmul free dim (one fp32 PSUM bank)
CHUNK = 2048     # max corpus columns per outer chunk == PSUM tile columns

F32 = mybir.dt.float32
BF16 = mybir.dt.bfloat16
ACTF = mybir.ActivationFunctionType


def build_nc(rows_per_core: int, corpus_rows: int) -> bass.Bass:
    # Bacc compile() splits multi-sem waits into event-semaphore
    # instructions where an instruction can carry only one wait.
    nc = bacc.Bacc(None)

    x = nc.dram_tensor("x", [rows_per_core, D], BF16, kind="ExternalInput")
    y = nc.dram_tensor("y", [corpus_rows, D], BF16, kind="ExternalInput")
    out = nc.dram_tensor(
        "out", [rows_per_core, corpus_rows], BF16, kind="ExternalOutput"
    )

    nbx = rows_per_core // P          # x row-blocks (8)
    # Small first chunk starts the store pipeline early; small last chunk
    # shortens the drain tail after the final matmul.
    if corpus_rows >= 4 * CHUNK:
        half = CHUNK // 2
        nfull = (corpus_rows - 2 * half) // CHUNK
        chunk_cols = [half] + [CHUNK] * nfull + [half]
    else:
        chunk_cols = [CHUNK] * (corpus_rows // CHUNK)
    assert sum(chunk_cols) == corpus_rows
    chunk_starts = [sum(chunk_cols[:i]) for i in range(len(chunk_cols))]

    with tile.TileContext(nc) as tc:
        with (
            tc.tile_pool(name="const", bufs=1) as constp,
            tc.tile_pool(name="xp", bufs=1) as xp,
            tc.tile_pool(name="yt", bufs=3) as ytp,
            tc.tile_pool(name="sq", bufs=2) as sqp,
            tc.tile_pool(name="in2", bufs=2) as in2p,
            tc.tile_pool(name="invb", bufs=2) as invbp,
            tc.tile_pool(name="yn", bufs=2) as ynp,
            tc.tile_pool(name="obuf", bufs=6) as obufp,
            tc.tile_pool(name="mm", bufs=2, space=bass.MemorySpace.PSUM) as mpsum,
        ):
            ones = constp.tile([P, P], BF16)
            nc.gpsimd.memset(ones[:], 1.0)

            wt = constp.tile([P, NT], BF16)
            nc.gpsimd.memset(wt[:], 0.0)

            # PE keep-warm: dummy bf16 matmul batches. PE-only deps, so the
            # PE burns through them whenever it would otherwise idle.
            def warm(n):
                wps = mpsum.tile([P, CHUNK], F32, tag="ps")
                for i in range(n):
                    nc.tensor.matmul(
                        wps[:, (i % 2) * NT : (i % 2) * NT + NT],
                        wt[:, :P],
                        wt[:],
                        start=True,
                        stop=True,
                    )

            # Transpose-load a [cols, D] row range of src into [128, cols]
            # bf16 (xbar; SP HWDGE ring, shared with stores).
            def load_chunk(src, r0, cols):
                tT = ytp.tile([P, CHUNK], BF16, tag="yt")
                nc.sync.dma_start(
                    out=tT[:, :cols], in_=src[r0 : r0 + cols, :], transpose=True
                )
                return tT

            # Column-normalize tT via the ones-matmul norm broadcast.
            def norm_chunk(tT, cols):
                sq = sqp.tile([P, CHUNK], BF16, tag="sq")
                nc.gpsimd.tensor_mul(sq[:, :cols], tT[:, :cols], tT[:, :cols])
                nps = mpsum.tile([P, CHUNK], F32, tag="ps")
                for j in range(0, cols, NT):
                    nc.tensor.matmul(
                        nps[:, j : j + NT],
                        ones[:],
                        sq[:, j : j + NT],
                        start=True,
                        stop=True,
                    )
                # 1/||t||^2 broadcast on every partition, fused PSUM drain.
                in2 = in2p.tile([P, CHUNK], F32, tag="in2")
                nc.vector.reciprocal_approx_fast(in2[:, :cols], nps[:, :cols])
                invb = invbp.tile([P, CHUNK], BF16, tag="invb")
                nc.scalar.sqrt(invb[:, :cols], in2[:, :cols])
                tn = ynp.tile([P, CHUNK], BF16, tag="yn")
                nc.gpsimd.tensor_mul(tn[:, :cols], tT[:, :cols], invb[:, :cols])
                return tn

            warm(12)
            xT = load_chunk(x[:], 0, rows_per_core)
            yT0 = load_chunk(y[:], 0, chunk_cols[0])
            xTn = norm_chunk(xT, rows_per_core)
            warm(10)
            yTn = norm_chunk(yT0, chunk_cols[0])
            warm(10)

            yT_next = None
            for c, cols in enumerate(chunk_cols):
                col0 = chunk_starts[c]
                has_next = c + 1 < len(chunk_cols)
                yTn_next = None
                for b in range(nbx):
                    if b == 0 and has_next:
                        # Next chunk's load+norm ride ahead of the drain flood.
                        yT_next = load_chunk(y[:], chunk_starts[c + 1], chunk_cols[c + 1])
                    if b == 1 and has_next:
                        yTn_next = norm_chunk(yT_next, chunk_cols[c + 1])
                    lhs = xTn[:, b * P : (b + 1) * P]
                    ob = obufp.tile([P, CHUNK], BF16, tag="ob")
                    ps = mpsum.tile([P, CHUNK], F32)
                    for j in range(0, cols, NT):
                        nc.tensor.matmul(
                            ps[:, j : j + NT],
                            lhs,
                            yTn[:, j : j + NT],
                            start=True,
                            stop=True,
                        )
                    # One 4-bank drain per block, whole block on one engine
                    # (alternating) -> single-producer stores, fewest
                    # instructions per output byte.
                    if b % 2 == 0:
                        nc.vector.tensor_copy(ob[:, :cols], ps[:, :cols])
                    else:
                        nc.scalar.copy(ob[:, :cols], ps[:, :cols])
                    nc.sync.dma_start(
                        out=out[b * P : (b + 1) * P, col0 : col0 + cols],
                        in_=ob[:, :cols],
                    )
                if yTn_next is not None:
                    yTn = yTn_next

    nc.finalize()
    return nc


_NC_CACHE: dict[tuple[int, int], bass.Bass] = {}


def run_spmd(input1: np.ndarray, input2: np.ndarray, **kwargs):
    """Shard, run on 8 cores, gather. Returns (output, BassKernelResults)."""
    x_bf = np.asarray(input1, dtype=np.float32).astype(ml_dtypes.bfloat16)
    y_bf = np.ascontiguousarray(
        np.asarray(input2, dtype=np.float32).astype(ml_dtypes.bfloat16)
    )
    n, d = x_bf.shape
    m, d2 = y_bf.shape
    assert d == D and d2 == D and n % N_CORES == 0
    rows = n // N_CORES

    key = (rows, m)
    if key not in _NC_CACHE:
        _NC_CACHE[key] = build_nc(rows, m)
    nc = _NC_CACHE[key]

    in_maps = [
        {"x": np.ascontiguousarray(x_bf[c * rows : (c + 1) * rows]), "y": y_bf}
        for c in range(N_CORES)
    ]
    res = run_bass_kernel_spmd(nc, in_maps, core_ids=list(range(N_CORES)), **kwargs)
    out = np.concatenate(
        [res.results[c]["out"].astype(np.float32) for c in range(N_CORES)], axis=0
    )
    return out, res


def kernel(input1: np.ndarray, input2: np.ndarray) -> np.ndarray:
    return run_spmd(input1, input2)[0]


# revision 15
# speedup vs baseline: 1.1470x; 1.1305x over previous
"""Pairwise cosine similarity on 8 Trainium2 NeuronCores.

Computes sim[n, m] = <x_n, y_m> / (||x_n|| * ||y_m||) for
input1 [8192, 128], input2 [8192, 128] -> out [8192, 8192] (fp32 API).

Sharding: input1 rows split 8 ways (1024 rows/core); input2 replicated.
Each core computes one [1024, 8192] output stripe; host concatenates.

Precision plan (checker budget: rel_err < 2e-2 vs absmax): host casts
inputs to bf16 (round-to-nearest) and upcasts the bf16 output stripe to
fp32. Device math: bf16 matmuls with fp32 PSUM accumulation; inv-norms
via reciprocal_approx_fast (~51 ULP) + ACT Sqrt. Measured ~7e-3 rel
err -- well inside budget -- and both HBM streams are 2-byte
(loads 2.25 MB + stores 16.8 MB per core ~= 53 us HBM floor).

Structure notes (why it looks the way it does):
- Operands arrive in [d, rows] layout via DMA xbar transpose on load --
  zero PE transposes, zero fp32 matmul operands (bf16 streams at
  1 row/cycle vs 2 for fp32).
- Norms come from a ones-matmul (broadcasts ||t||^2 down the partition
  axis), a DVE reciprocal_approx drain, an ACT sqrt (fused fp32->bf16
  cast), and one GpSimd multiply -- nothing on the critical drain queues.
- PSUM tiles are 4 banks ([128, 2048] fp32) and each output block is
  drained by a single engine (DVE/ACT alternating per block): the big
  tiles halve instruction count, and single-producer stores + single-dep
  matmuls minimize Bacc's event-semaphore wait-splitting, which
  otherwise floods every queue at ~0.3 us per handoff.
- Dummy PE matmul batches bridge the prep phases so the HAM clock gate
  stays open (1.2 vs 2.4 GHz) into the real matmul stream.
"""

import numpy as np
import ml_dtypes

import concourse.bass as bass
import concourse.tile as tile
from concourse import bacc, mybir
from concourse.bass_utils import run_bass_kernel_spmd

N_CORES = 8
D = 128          # feature dim == contraction partitions
P = 128          # SBUF partitions
NT = 512         # matmul free dim (one fp32 PSUM bank)
CHUNK = 2048     # max corpus columns per outer chunk
MMCOLS = 1024    # PSUM tile columns (2 banks; pool of 4 => all 8 banks)

F32 = mybir.dt.float32
BF16 = mybir.dt.bfloat16
ACTF = mybir.ActivationFunctionType


def build_nc(rows_per_core: int, corpus_rows: int) -> bass.Bass:
    # Bacc compile() splits multi-sem waits into event-semaphore
    # instructions where an instruction can carry only one wait.
    nc = bacc.Bacc(None)

    x = nc.dram_tensor("x", [rows_per_core, D], BF16, kind="ExternalInput")
    y = nc.dram_tensor("y", [corpus_rows, D], BF16, kind="ExternalInput")
    out = nc.dram_tensor(
        "out", [rows_per_core, corpus_rows], BF16, kind="ExternalOutput"
    )

    nbx = rows_per_core // P          # x row-blocks (8)
    # Small first chunk starts the store pipeline early; small last chunk
    # shortens the drain tail after the final matmul.
    if corpus_rows >= 4 * CHUNK:
        half = CHUNK // 2
        nfull = (corpus_rows - 2 * half) // CHUNK
        chunk_cols = [half] + [CHUNK] * nfull + [half]
    else:
        chunk_cols = [CHUNK] * (corpus_rows // CHUNK)
    assert sum(chunk_cols) == corpus_rows
    chunk_starts = [sum(chunk_cols[:i]) for i in range(len(chunk_cols))]

    with tile.TileContext(nc) as tc:
        with (
            tc.tile_pool(name="const", bufs=1) as constp,
            tc.tile_pool(name="xp", bufs=1) as xp,
            tc.tile_pool(name="yt", bufs=3) as ytp,
            tc.tile_pool(name="sq", bufs=2) as sqp,
            tc.tile_pool(name="in2", bufs=2) as in2p,
            tc.tile_pool(name="invb", bufs=2) as invbp,
            tc.tile_pool(name="yn", bufs=2) as ynp,
            tc.tile_pool(name="obuf", bufs=6) as obufp,
            tc.tile_pool(name="mm", bufs=4, space=bass.MemorySpace.PSUM) as mpsum,
        ):
            ones = constp.tile([P, P], BF16)
            nc.gpsimd.memset(ones[:], 1.0)

            wt = constp.tile([P, NT], BF16)
            nc.gpsimd.memset(wt[:], 0.0)

            # PE keep-warm: dummy bf16 matmul batches. PE-only deps, so the
            # PE burns through them whenever it would otherwise idle.
            def warm(n):
                wps = mpsum.tile([P, MMCOLS], F32, tag="ps")
                for i in range(n):
                    nc.tensor.matmul(
                        wps[:, (i % 2) * NT : (i % 2) * NT + NT],
                        wt[:, :P],
                        wt[:],
                        start=True,
                        stop=True,
                    )

            # Transpose-load a [cols, D] row range of src into [128, cols]
            # bf16 (xbar; SP HWDGE ring, shared with stores).
            def load_chunk(src, r0, cols):
                tT = ytp.tile([P, CHUNK], BF16, tag="yt")
                nc.scalar.dma_start(
                    out=tT[:, :cols], in_=src[r0 : r0 + cols, :], transpose=True
                )
                return tT

            # Column-normalize tT via the ones-matmul norm broadcast.
            def norm_chunk(tT, cols):
                sq = sqp.tile([P, CHUNK], BF16, tag="sq")
                nc.gpsimd.tensor_mul(sq[:, :cols], tT[:, :cols], tT[:, :cols])
                in2 = in2p.tile([P, CHUNK], F32, tag="in2")
                for h in range(0, cols, MMCOLS):
                    hc = min(MMCOLS, cols - h)
                    nps = mpsum.tile([P, MMCOLS], F32, tag="ps")
                    for j in range(0, hc, NT):
                        nc.tensor.matmul(
                            nps[:, j : j + NT],
                            ones[:],
                            sq[:, h + j : h + j + NT],
                            start=True,
                            stop=True,
                        )
                    # 1/||t||^2 broadcast on every partition, fused PSUM drain.
                    nc.vector.reciprocal_approx_fast(
                        in2[:, h : h + hc], nps[:, :hc]
                    )
                invb = invbp.tile([P, CHUNK], BF16, tag="invb")
                nc.scalar.sqrt(invb[:, :cols], in2[:, :cols])
                tn = ynp.tile([P, CHUNK], BF16, tag="yn")
                nc.gpsimd.tensor_mul(tn[:, :cols], tT[:, :cols], invb[:, :cols])
                return tn

            warm(12)
            xT = load_chunk(x[:], 0, rows_per_core)
            yT0 = load_chunk(y[:], 0, chunk_cols[0])
            xTn = norm_chunk(xT, rows_per_core)
            warm(10)
            yTn = norm_chunk(yT0, chunk_cols[0])
            warm(10)

            yT_next = None
            for c, cols in enumerate(chunk_cols):
                col0 = chunk_starts[c]
                has_next = c + 1 < len(chunk_cols)
                yTn_next = None
                for b in range(nbx):
                    if b == 0 and has_next:
                        # Next chunk's load+norm ride ahead of the drain flood.
                        yT_next = load_chunk(y[:], chunk_starts[c + 1], chunk_cols[c + 1])
                    if b == 1 and has_next:
                        yTn_next = norm_chunk(yT_next, chunk_cols[c + 1])
                    lhs = xTn[:, b * P : (b + 1) * P]
                    ob = obufp.tile([P, CHUNK], BF16, tag="ob")
                    for h in range(0, cols, MMCOLS):
                        hc = min(MMCOLS, cols - h)
                        ps = mpsum.tile([P, MMCOLS], F32, tag="ps")
                        for j in range(0, hc, NT):
                            nc.tensor.matmul(
                                ps[:, j : j + NT],
                                lhs,
                                yTn[:, h + j : h + j + NT],
                                start=True,
                                stop=True,
                            )
                        # Whole block drains on one engine (alternating per
                        # block) -> stores wait on a single producer.
                        if b % 2 == 0:
                            nc.vector.tensor_copy(ob[:, h : h + hc], ps[:, :hc])
                        else:
                            nc.scalar.copy(ob[:, h : h + hc], ps[:, :hc])
                    nc.sync.dma_start(
                        out=out[b * P : (b + 1) * P, col0 : col0 + cols],
                        in_=ob[:, :cols],
                    )
                if yTn_next is not None:
                    yTn = yTn_next

    nc.finalize()
    return nc


_NC_CACHE: dict[tuple[int, int], bass.Bass] = {}


def run_spmd(input1: np.ndarray, input2: np.ndarray, **kwargs):
    """Shard, run on 8 cores, gather. Returns (output, BassKernelResults)."""
    x_bf = np.asarray(input1, dtype=np.float32).astype(ml_dtypes.bfloat16)
    y_bf = np.ascontiguousarray(
        np.asarray(input2, dtype=np.float32).astype(ml_dtypes.bfloat16)
    )
    n, d = x_bf.shape
    m, d2 = y_bf.shape
    assert d == D and d2 == D and n % N_CORES == 0
    rows = n // N_CORES

    key = (rows, m)
    if key not in _NC_CACHE:
        _NC_CACHE[key] = build_nc(rows, m)
    nc = _NC_CACHE[key]

    in_maps = [
        {"x": np.ascontiguousarray(x_bf[c * rows : (c + 1) * rows]), "y": y_bf}
        for c in range(N_CORES)
    ]
    res = run_bass_kernel_spmd(nc, in_maps, core_ids=list(range(N_CORES)), **kwargs)
    out = np.concatenate(
        [res.results[c]["out"].astype(np.float32) for c in range(N_CORES)], axis=0
    )
    return out, res


def kernel(input1: np.ndarray, input2: np.ndarray) -> np.ndarray:
    return run_spmd(input1, input2)[0]


# revision 18
# speedup vs baseline: 1.1559x; 1.0078x over previous
"""Pairwise cosine similarity on 8 Trainium2 NeuronCores.

Computes sim[n, m] = <x_n, y_m> / (||x_n|| * ||y_m||) for
input1 [8192, 128], input2 [8192, 128] -> out [8192, 8192] (fp32 API).

Sharding: input1 rows split 8 ways (1024 rows/core); input2 replicated.
Each core computes one [1024, 8192] output stripe; host concatenates.

Precision plan (checker budget: rel_err < 2e-2 vs absmax): host casts
inputs to bf16 (round-to-nearest) and upcasts the bf16 output stripe to
fp32. Device math: bf16 matmuls with fp32 PSUM accumulation; inv-norms
via reciprocal_approx_fast (~51 ULP) + ACT Sqrt. Measured ~7e-3 rel
err -- well inside budget -- and both HBM streams are 2-byte
(loads 2.25 MB + stores 16.8 MB per core ~= 53 us HBM floor).

Structure notes (why it looks the way it does):
- Operands arrive in [d, rows] layout via DMA xbar transpose on load --
  zero PE transposes, zero fp32 matmul operands (bf16 streams at
  1 row/cycle vs 2 for fp32).
- Norms come from a ones-matmul (broadcasts ||t||^2 down the partition
  axis), a DVE reciprocal_approx drain, an ACT sqrt (fused fp32->bf16
  cast), and one GpSimd multiply -- nothing on the critical drain queues.
- PSUM tiles are 4 banks ([128, 2048] fp32) and each output block is
  drained by a single engine (DVE/ACT alternating per block): the big
  tiles halve instruction count, and single-producer stores + single-dep
  matmuls minimize Bacc's event-semaphore wait-splitting, which
  otherwise floods every queue at ~0.3 us per handoff.
- Dummy PE matmul batches bridge the prep phases so the HAM clock gate
  stays open (1.2 vs 2.4 GHz) into the real matmul stream.
"""

import numpy as np
import ml_dtypes

import concourse.bass as bass
import concourse.tile as tile
from concourse import bacc, mybir
from concourse.bass_utils import run_bass_kernel_spmd

N_CORES = 8
D = 128          # feature dim == contraction partitions
P = 128          # SBUF partitions
NT = 512         # matmul free dim (one fp32 PSUM bank)
CHUNK = 2048     # max corpus columns per outer chunk
MMCOLS = 1024    # PSUM tile columns (2 banks; pool of 4 => all 8 banks)

F32 = mybir.dt.float32
BF16 = mybir.dt.bfloat16
ACTF = mybir.ActivationFunctionType


def build_nc(rows_per_core: int, corpus_rows: int) -> bass.Bass:
    # Bacc compile() splits multi-sem waits into event-semaphore
    # instructions where an instruction can carry only one wait.
    nc = bacc.Bacc(None)

    x = nc.dram_tensor("x", [rows_per_core, D], BF16, kind="ExternalInput")
    y = nc.dram_tensor("y", [corpus_rows, D], BF16, kind="ExternalInput")
    out = nc.dram_tensor(
        "out", [rows_per_core, corpus_rows], BF16, kind="ExternalOutput"
    )

    nbx = rows_per_core // P          # x row-blocks (8)
    # Small first chunk starts the store pipeline early; small last chunk
    # shortens the drain tail after the final matmul.
    if corpus_rows >= 4 * CHUNK:
        half = CHUNK // 2
        nfull = (corpus_rows - 2 * half) // CHUNK
        chunk_cols = [half] + [CHUNK] * nfull + [half]
    else:
        chunk_cols = [CHUNK] * (corpus_rows // CHUNK)
    assert sum(chunk_cols) == corpus_rows
    chunk_starts = [sum(chunk_cols[:i]) for i in range(len(chunk_cols))]

    with tile.TileContext(nc) as tc:
        with (
            tc.tile_pool(name="const", bufs=1) as constp,
            tc.tile_pool(name="xp", bufs=1) as xp,
            tc.tile_pool(name="yt", bufs=3) as ytp,
            tc.tile_pool(name="sq", bufs=2) as sqp,
            tc.tile_pool(name="in2", bufs=2) as in2p,
            tc.tile_pool(name="invb", bufs=2) as invbp,
            tc.tile_pool(name="yn", bufs=2) as ynp,
            tc.tile_pool(name="obuf", bufs=6) as obufp,
            tc.tile_pool(name="mm", bufs=4, space=bass.MemorySpace.PSUM) as mpsum,
        ):
            ones = constp.tile([P, P], BF16)
            nc.gpsimd.memset(ones[:], 1.0)

            wt = constp.tile([P, NT], BF16)
            nc.gpsimd.memset(wt[:], 0.0)

            # PE keep-warm: dummy bf16 matmul batches. PE-only deps, so the
            # PE burns through them whenever it would otherwise idle.
            def warm(n):
                wps = mpsum.tile([P, MMCOLS], F32, tag="ps")
                for i in range(n):
                    nc.tensor.matmul(
                        wps[:, (i % 2) * NT : (i % 2) * NT + NT],
                        wt[:, :P],
                        wt[:],
                        start=True,
                        stop=True,
                    )

            # Transpose-load a [cols, D] row range of src into [128, cols]
            # bf16 (xbar; SP HWDGE ring, shared with stores).
            def load_chunk(src, r0, cols):
                tT = ytp.tile([P, CHUNK], BF16, tag="yt")
                nc.scalar.dma_start(
                    out=tT[:, :cols], in_=src[r0 : r0 + cols, :], transpose=True
                )
                return tT

            # Column-normalize tT via the ones-matmul norm broadcast.
            # fast=True runs the square/scale TTs on DVE (bf16 2x mode,
            # ~4x quicker than GpSimd) -- used during the ramp, when DVE
            # has no drain work yet and GpSimd would serialize the chains.
            def norm_chunk(tT, cols, fast=False):
                tt_eng = nc.vector if fast else nc.gpsimd
                sq = sqp.tile([P, CHUNK], BF16, tag="sq")
                tt_eng.tensor_mul(sq[:, :cols], tT[:, :cols], tT[:, :cols])
                in2 = in2p.tile([P, CHUNK], F32, tag="in2")
                for h in range(0, cols, MMCOLS):
                    hc = min(MMCOLS, cols - h)
                    nps = mpsum.tile([P, MMCOLS], F32, tag="ps")
                    for j in range(0, hc, NT):
                        nc.tensor.matmul(
                            nps[:, j : j + NT],
                            ones[:],
                            sq[:, h + j : h + j + NT],
                            start=True,
                            stop=True,
                        )
                    # 1/||t||^2 broadcast on every partition, fused PSUM drain.
                    nc.vector.reciprocal_approx_fast(
                        in2[:, h : h + hc], nps[:, :hc]
                    )
                invb = invbp.tile([P, CHUNK], BF16, tag="invb")
                nc.scalar.sqrt(invb[:, :cols], in2[:, :cols])
                tn = ynp.tile([P, CHUNK], BF16, tag="yn")
                tt_eng.tensor_mul(tn[:, :cols], tT[:, :cols], invb[:, :cols])
                return tn

            warm(12)
            xT = load_chunk(x[:], 0, rows_per_core)
            yT0 = load_chunk(y[:], 0, chunk_cols[0])
            xTn = norm_chunk(xT, rows_per_core, fast=True)
            warm(10)
            yTn = norm_chunk(yT0, chunk_cols[0], fast=True)
            warm(10)

            yT_next = None
            for c, cols in enumerate(chunk_cols):
                col0 = chunk_starts[c]
                has_next = c + 1 < len(chunk_cols)
                yTn_next = None
                for b in range(nbx):
                    if b == 0 and has_next:
                        # Next chunk's load+norm ride ahead of the drain flood.
                        yT_next = load_chunk(y[:], chunk_starts[c + 1], chunk_cols[c + 1])
                    if b == 1 and has_next:
                        yTn_next = norm_chunk(yT_next, chunk_cols[c + 1])
                    lhs = xTn[:, b * P : (b + 1) * P]
                    ob = obufp.tile([P, CHUNK], BF16, tag="ob")
                    for h in range(0, cols, MMCOLS):
                        hc = min(MMCOLS, cols - h)
                        ps = mpsum.tile([P, MMCOLS], F32, tag="ps")
                        for j in range(0, hc, NT):
                            nc.tensor.matmul(
                                ps[:, j : j + NT],
                                lhs,
                                yTn[:, h + j : h + j + NT],
                                start=True,
                                stop=True,
                            )
                        # Whole block drains on one engine (alternating per
                        # block) -> stores wait on a single producer.
                        if b % 2 == 0:
                            nc.vector.tensor_copy(ob[:, h : h + hc], ps[:, :hc])
                        else:
                            nc.scalar.copy(ob[:, h : h + hc], ps[:, :hc])
                    nc.sync.dma_start(
                        out=out[b * P : (b + 1) * P, col0 : col0 + cols],
                        in_=ob[:, :cols],
                    )
                if yTn_next is not None:
                    yTn = yTn_next

    nc.finalize()
    return nc


_NC_CACHE: dict[tuple[int, int], bass.Bass] = {}


def run_spmd(input1: np.ndarray, input2: np.ndarray, **kwargs):
    """Shard, run on 8 cores, gather. Returns (output, BassKernelResults)."""
    x_bf = np.asarray(input1, dtype=np.float32).astype(ml_dtypes.bfloat16)
    y_bf = np.ascontiguousarray(
        np.asarray(input2, dtype=np.float32).astype(ml_dtypes.bfloat16)
    )
    n, d = x_bf.shape
    m, d2 = y_bf.shape
    assert d == D and d2 == D and n % N_CORES == 0
    rows = n // N_CORES

    key = (rows, m)
    if key not in _NC_CACHE:
        _NC_CACHE[key] = build_nc(rows, m)
    nc = _NC_CACHE[key]

    in_maps = [
        {"x": np.ascontiguousarray(x_bf[c * rows : (c + 1) * rows]), "y": y_bf}
        for c in range(N_CORES)
    ]
    res = run_bass_kernel_spmd(nc, in_maps, core_ids=list(range(N_CORES)), **kwargs)
    out = np.concatenate(
        [res.results[c]["out"].astype(np.float32) for c in range(N_CORES)], axis=0
    )
    return out, res


def kernel(input1: np.ndarray, input2: np.ndarray) -> np.ndarray:
    return run_spmd(input1, input2)[0]


# revision 23
# speedup vs baseline: 1.3204x; 1.1423x over previous
"""Pairwise cosine similarity on 8 Trainium2 NeuronCores.

Computes sim[n, m] = <x_n, y_m> / (||x_n|| * ||y_m||) for
input1 [8192, 128], input2 [8192, 128] -> out [8192, 8192] (fp32 API).

Sharding: input1 rows split 8 ways (1024 rows/core); input2 replicated.
Each core computes one [1024, 8192] output stripe; host concatenates.

Precision plan (checker budget: rel_err < 2e-2 vs absmax): host casts
inputs to bf16 (round-to-nearest) and upcasts the bf16 output stripe to
fp32. Device math: bf16 matmuls with fp32 PSUM accumulation; inv-norms
via reciprocal_approx_fast (~51 ULP) + ACT Sqrt. Measured ~7e-3 rel
err -- well inside budget -- and both HBM streams are 2-byte
(loads 2.25 MB + stores 16.8 MB per core ~= 53 us HBM floor).

Structure notes (why it looks the way it does):
- Operands arrive in [d, rows] layout via DMA xbar transpose on load --
  zero PE transposes, zero fp32 matmul operands (bf16 streams at
  1 row/cycle vs 2 for fp32).
- Norms come from a ones-matmul (broadcasts ||t||^2 down the partition
  axis), a DVE reciprocal_approx drain, an ACT sqrt (fused fp32->bf16
  cast), and one GpSimd multiply -- nothing on the critical drain queues.
- PSUM tiles are 4 banks ([128, 2048] fp32) and each output block is
  drained by a single engine (DVE/ACT alternating per block): the big
  tiles halve instruction count, and single-producer stores + single-dep
  matmuls minimize Bacc's event-semaphore wait-splitting, which
  otherwise floods every queue at ~0.3 us per handoff.
- Dummy PE matmul batches bridge the prep phases so the HAM clock gate
  stays open (1.2 vs 2.4 GHz) into the real matmul stream.
"""

import numpy as np
import ml_dtypes

import concourse.bass as bass
import concourse.tile as tile
from concourse import bacc, mybir
from concourse.bass_utils import run_bass_kernel_spmd

N_CORES = 8
D = 128          # feature dim == contraction partitions
P = 128          # SBUF partitions
NT = 512         # matmul free dim (one fp32 PSUM bank)
CHUNK = 2048     # max corpus columns per outer chunk
MMCOLS = 1024    # PSUM tile columns (2 banks; pool of 4 => all 8 banks)

F32 = mybir.dt.float32
BF16 = mybir.dt.bfloat16
ACTF = mybir.ActivationFunctionType


def build_nc(rows_per_core: int, corpus_rows: int) -> bass.Bass:
    # Bacc compile() splits multi-sem waits into event-semaphore
    # instructions where an instruction can carry only one wait.
    nc = bacc.Bacc(None)

    x = nc.dram_tensor("x", [rows_per_core, D], BF16, kind="ExternalInput")
    y = nc.dram_tensor("y", [corpus_rows, D], BF16, kind="ExternalInput")
    out = nc.dram_tensor(
        "out", [rows_per_core, corpus_rows], BF16, kind="ExternalOutput"
    )

    nbx = rows_per_core // P          # x row-blocks (8)
    # Small first chunk starts the store pipeline early; small last chunk
    # shortens the drain tail after the final matmul.
    if corpus_rows >= 4 * CHUNK:
        half = CHUNK // 2
        nfull = (corpus_rows - 2 * half) // CHUNK
        chunk_cols = [half] + [CHUNK] * nfull + [half]
    else:
        chunk_cols = [CHUNK] * (corpus_rows // CHUNK)
    assert sum(chunk_cols) == corpus_rows
    chunk_starts = [sum(chunk_cols[:i]) for i in range(len(chunk_cols))]

    with tile.TileContext(nc) as tc:
        with (
            tc.tile_pool(name="const", bufs=1) as constp,
            tc.tile_pool(name="xn", bufs=1) as xnp,
            tc.tile_pool(name="yt", bufs=4) as ytp,
            tc.tile_pool(name="sq", bufs=3) as sqp,
            tc.tile_pool(name="in2", bufs=3) as in2p,
            tc.tile_pool(name="invb", bufs=3) as invbp,
            tc.tile_pool(name="yn", bufs=3) as ynp,
            tc.tile_pool(name="obuf", bufs=6) as obufp,
            tc.tile_pool(name="mm", bufs=4, space=bass.MemorySpace.PSUM) as mpsum,
        ):
            ones = constp.tile([P, P], BF16)
            nc.gpsimd.memset(ones[:], 1.0)

            wt = constp.tile([P, NT], BF16)
            nc.gpsimd.memset(wt[:], 0.0)

            # PE keep-warm: dummy bf16 matmul batches. PE-only deps, so the
            # PE burns through them whenever it would otherwise idle.
            def warm(n):
                wps = mpsum.tile([P, MMCOLS], F32, tag="ps")
                for i in range(n):
                    nc.tensor.matmul(
                        wps[:, (i % 2) * NT : (i % 2) * NT + NT],
                        wt[:, :P],
                        wt[:],
                        start=True,
                        stop=True,
                    )

            # Transpose-load a [cols, D] row range of src into [128, cols]
            # bf16 (xbar; SP HWDGE ring, shared with stores).
            def load_chunk(src, r0, cols):
                tT = ytp.tile([P, CHUNK], BF16, tag="yt")
                nc.scalar.dma_start(
                    out=tT[:, :cols], in_=src[r0 : r0 + cols, :], transpose=True
                )
                return tT

            # Column-normalize tT via the ones-matmul norm broadcast.
            # fast=True runs the square/scale TTs on DVE (bf16 2x mode,
            # ~4x quicker than GpSimd) -- used during the ramp, when DVE
            # has no drain work yet and GpSimd would serialize the chains.
            def norm_chunk(tT, cols, fast=False, pool=None):
                tt_eng = nc.vector if fast else nc.gpsimd
                sq = sqp.tile([P, CHUNK], BF16, tag="sq")
                tt_eng.tensor_mul(sq[:, :cols], tT[:, :cols], tT[:, :cols])
                in2 = in2p.tile([P, CHUNK], F32, tag="in2")
                for h in range(0, cols, MMCOLS):
                    hc = min(MMCOLS, cols - h)
                    nps = mpsum.tile([P, MMCOLS], F32, tag="ps")
                    for j in range(0, hc, NT):
                        nc.tensor.matmul(
                            nps[:, j : j + NT],
                            ones[:],
                            sq[:, h + j : h + j + NT],
                            start=True,
                            stop=True,
                        )
                    # 1/||t||^2 broadcast on every partition, fused PSUM drain.
                    nc.vector.reciprocal_approx_fast(
                        in2[:, h : h + hc], nps[:, :hc]
                    )
                invb = invbp.tile([P, CHUNK], BF16, tag="invb")
                nc.scalar.sqrt(invb[:, :cols], in2[:, :cols])
                tn = (pool or ynp).tile([P, CHUNK], BF16, tag="tn")
                tt_eng.tensor_mul(tn[:, :cols], tT[:, :cols], invb[:, :cols])
                return tn

            nchunk = len(chunk_cols)
            warm(12)
            # Prefetch depth 2: chunk c+2's load+norm are issued during
            # chunk c, so the ~10us norm-chain latency never gaps the
            # store stream at a chunk boundary.
            xT = load_chunk(x[:], 0, rows_per_core)
            yT = {0: load_chunk(y[:], 0, chunk_cols[0])}
            if nchunk > 1:
                yT[1] = load_chunk(y[:], chunk_starts[1], chunk_cols[1])
            xTn = norm_chunk(xT, rows_per_core, fast=True, pool=xnp)
            yTn_d = {0: norm_chunk(yT[0], chunk_cols[0], fast=True)}
            warm(8)
            if nchunk > 1:
                yTn_d[1] = norm_chunk(yT[1], chunk_cols[1], fast=True)

            for c, cols in enumerate(chunk_cols):
                col0 = chunk_starts[c]
                yTn = yTn_d.pop(c)
                for b in range(nbx):
                    if b == 0 and c + 2 < nchunk:
                        yT[c + 2] = load_chunk(
                            y[:], chunk_starts[c + 2], chunk_cols[c + 2]
                        )
                    if b == 1 and c + 2 < nchunk:
                        yTn_d[c + 2] = norm_chunk(yT[c + 2], chunk_cols[c + 2])
                    lhs = xTn[:, b * P : (b + 1) * P]
                    ob = obufp.tile([P, CHUNK], BF16, tag="ob")
                    for h in range(0, cols, MMCOLS):
                        hc = min(MMCOLS, cols - h)
                        ps = mpsum.tile([P, MMCOLS], F32, tag="ps")
                        for j in range(0, hc, NT):
                            nc.tensor.matmul(
                                ps[:, j : j + NT],
                                lhs,
                                yTn[:, h + j : h + j + NT],
                                start=True,
                                stop=True,
                            )
                        # Whole block drains on one engine (alternating per
                        # block) -> stores wait on a single producer.
                        if b % 2 == 0:
                            nc.vector.tensor_copy(ob[:, h : h + hc], ps[:, :hc])
                        else:
                            nc.scalar.copy(ob[:, h : h + hc], ps[:, :hc])
                    nc.sync.dma_start(
                        out=out[b * P : (b + 1) * P, col0 : col0 + cols],
                        in_=ob[:, :cols],
                    )

    nc.finalize()
    return nc


_NC_CACHE: dict[tuple[int, int], bass.Bass] = {}


def run_spmd(input1: np.ndarray, input2: np.ndarray, **kwargs):
    """Shard, run on 8 cores, gather. Returns (output, BassKernelResults)."""
    x_bf = np.asarray(input1, dtype=np.float32).astype(ml_dtypes.bfloat16)
    y_bf = np.ascontiguousarray(
        np.asarray(input2, dtype=np.float32).astype(ml_dtypes.bfloat16)
    )
    n, d = x_bf.shape
    m, d2 = y_bf.shape
    assert d == D and d2 == D and n % N_CORES == 0
    rows = n // N_CORES

    key = (rows, m)
    if key not in _NC_CACHE:
        _NC_CACHE[key] = build_nc(rows, m)
    nc = _NC_CACHE[key]

    in_maps = [
        {"x": np.ascontiguousarray(x_bf[c * rows : (c + 1) * rows]), "y": y_bf}
        for c in range(N_CORES)
    ]
    res = run_bass_kernel_spmd(nc, in_maps, core_ids=list(range(N_CORES)), **kwargs)
    out = np.concatenate(
        [res.results[c]["out"].astype(np.float32) for c in range(N_CORES)], axis=0
    )
    return out, res


def kernel(input1: np.ndarray, input2: np.ndarray) -> np.ndarray:
    return run_spmd(input1, input2)[0]


# revision 26
# speedup vs baseline: 1.3346x; 1.0108x over previous
"""Pairwise cosine similarity on 8 Trainium2 NeuronCores.

Computes sim[n, m] = <x_n, y_m> / (||x_n|| * ||y_m||) for
input1 [8192, 128], input2 [8192, 128] -> out [8192, 8192] (fp32 API).

Sharding: input1 rows split 8 ways (1024 rows/core); input2 replicated.
Each core computes one [1024, 8192] output stripe; host concatenates.

Precision plan (checker budget: rel_err < 2e-2 vs absmax): host casts
inputs to bf16 (round-to-nearest) and upcasts the bf16 output stripe to
fp32. Device math: bf16 matmuls with fp32 PSUM accumulation; inv-norms
via reciprocal_approx_fast (~51 ULP) + ACT Sqrt. Measured ~7e-3 rel
err -- well inside budget -- and both HBM streams are 2-byte
(loads 2.25 MB + stores 16.8 MB per core ~= 53 us HBM floor).

Structure notes (why it looks the way it does):
- Operands arrive in [d, rows] layout via DMA xbar transpose on load --
  zero PE transposes, zero fp32 matmul operands (bf16 streams at
  1 row/cycle vs 2 for fp32).
- Norms come from a ones-matmul (broadcasts ||t||^2 down the partition
  axis), a DVE reciprocal_approx drain, an ACT sqrt (fused fp32->bf16
  cast), and one GpSimd multiply -- nothing on the critical drain queues.
- PSUM tiles are 4 banks ([128, 2048] fp32) and each output block is
  drained by a single engine (DVE/ACT alternating per block): the big
  tiles halve instruction count, and single-producer stores + single-dep
  matmuls minimize Bacc's event-semaphore wait-splitting, which
  otherwise floods every queue at ~0.3 us per handoff.
- Dummy PE matmul batches bridge the prep phases so the HAM clock gate
  stays open (1.2 vs 2.4 GHz) into the real matmul stream.
"""

import numpy as np
import ml_dtypes

import concourse.bass as bass
import concourse.tile as tile
from concourse import bacc, mybir
from concourse.bass_utils import run_bass_kernel_spmd

N_CORES = 8
D = 128          # feature dim == contraction partitions
P = 128          # SBUF partitions
NT = 512         # matmul free dim (one fp32 PSUM bank)
CHUNK = 2048     # max corpus columns per outer chunk
MMCOLS = 1024    # PSUM tile columns (2 banks; pool of 4 => all 8 banks)

F32 = mybir.dt.float32
BF16 = mybir.dt.bfloat16
ACTF = mybir.ActivationFunctionType


def build_nc(rows_per_core: int, corpus_rows: int) -> bass.Bass:
    # Bacc compile() splits multi-sem waits into event-semaphore
    # instructions where an instruction can carry only one wait.
    nc = bacc.Bacc(None)

    x = nc.dram_tensor("x", [rows_per_core, D], BF16, kind="ExternalInput")
    y = nc.dram_tensor("y", [corpus_rows, D], BF16, kind="ExternalInput")
    out = nc.dram_tensor(
        "out", [rows_per_core, corpus_rows], BF16, kind="ExternalOutput"
    )

    nbx = rows_per_core // P          # x row-blocks (8)
    # Tiny first chunks get the store pipeline flowing during the ramp;
    # the rest run at full width.
    if corpus_rows >= 4 * CHUNK:
        nfull = (corpus_rows - 2 * CHUNK) // CHUNK
        chunk_cols = [CHUNK // 4, 3 * CHUNK // 4, CHUNK] + [CHUNK] * nfull
    else:
        chunk_cols = [CHUNK] * (corpus_rows // CHUNK)
    assert sum(chunk_cols) == corpus_rows
    chunk_starts = [sum(chunk_cols[:i]) for i in range(len(chunk_cols))]

    with tile.TileContext(nc) as tc:
        with (
            tc.tile_pool(name="const", bufs=1) as constp,
            tc.tile_pool(name="xn", bufs=1) as xnp,
            tc.tile_pool(name="yt", bufs=4) as ytp,
            tc.tile_pool(name="sq", bufs=3) as sqp,
            tc.tile_pool(name="in2", bufs=3) as in2p,
            tc.tile_pool(name="invb", bufs=3) as invbp,
            tc.tile_pool(name="yn", bufs=3) as ynp,
            tc.tile_pool(name="obuf", bufs=6) as obufp,
            tc.tile_pool(name="mm", bufs=4, space=bass.MemorySpace.PSUM) as mpsum,
        ):
            ones = constp.tile([P, P], BF16)
            nc.gpsimd.memset(ones[:], 1.0)

            wt = constp.tile([P, NT], BF16)
            nc.gpsimd.memset(wt[:], 0.0)

            # PE keep-warm: dummy bf16 matmul batches. PE-only deps, so the
            # PE burns through them whenever it would otherwise idle.
            def warm(n):
                wps = mpsum.tile([P, MMCOLS], F32, tag="ps")
                for i in range(n):
                    nc.tensor.matmul(
                        wps[:, (i % 2) * NT : (i % 2) * NT + NT],
                        wt[:, :P],
                        wt[:],
                        start=True,
                        stop=True,
                    )

            # Transpose-load a [cols, D] row range of src into [128, cols]
            # bf16 (xbar; SP HWDGE ring, shared with stores).
            def load_chunk(src, r0, cols):
                tT = ytp.tile([P, CHUNK], BF16, tag="yt")
                nc.scalar.dma_start(
                    out=tT[:, :cols], in_=src[r0 : r0 + cols, :], transpose=True
                )
                return tT

            # Column-normalize tT via the ones-matmul norm broadcast.
            # fast=True runs the square/scale TTs on DVE (bf16 2x mode,
            # ~4x quicker than GpSimd) -- used during the ramp, when DVE
            # has no drain work yet and GpSimd would serialize the chains.
            def norm_chunk(tT, cols, fast=False, pool=None):
                tt_eng = nc.vector if fast else nc.gpsimd
                sq = sqp.tile([P, CHUNK], BF16, tag="sq")
                tt_eng.tensor_mul(sq[:, :cols], tT[:, :cols], tT[:, :cols])
                in2 = in2p.tile([P, CHUNK], F32, tag="in2")
                for h in range(0, cols, MMCOLS):
                    hc = min(MMCOLS, cols - h)
                    nps = mpsum.tile([P, MMCOLS], F32, tag="ps")
                    for j in range(0, hc, NT):
                        nc.tensor.matmul(
                            nps[:, j : j + NT],
                            ones[:],
                            sq[:, h + j : h + j + NT],
                            start=True,
                            stop=True,
                        )
                    # 1/||t||^2 broadcast on every partition, fused PSUM drain.
                    nc.vector.reciprocal_approx_fast(
                        in2[:, h : h + hc], nps[:, :hc]
                    )
                invb = invbp.tile([P, CHUNK], BF16, tag="invb")
                nc.scalar.sqrt(invb[:, :cols], in2[:, :cols])
                tn = (pool or ynp).tile([P, CHUNK], BF16, tag="tn")
                tt_eng.tensor_mul(tn[:, :cols], tT[:, :cols], invb[:, :cols])
                return tn

            nchunk = len(chunk_cols)
            warm(8)
            # Prefetch depth 2: chunk c+2's load+norm are issued during
            # chunk c, so the ~10us norm-chain latency never gaps the
            # store stream at a chunk boundary.
            xT = load_chunk(x[:], 0, rows_per_core)
            yT = {0: load_chunk(y[:], 0, chunk_cols[0])}
            if nchunk > 1:
                yT[1] = load_chunk(y[:], chunk_starts[1], chunk_cols[1])
            xTn = norm_chunk(xT, rows_per_core, fast=True, pool=xnp)
            yTn_d = {0: norm_chunk(yT[0], chunk_cols[0], fast=True)}
            warm(6)
            if nchunk > 1:
                yTn_d[1] = norm_chunk(yT[1], chunk_cols[1], fast=True)

            for c, cols in enumerate(chunk_cols):
                col0 = chunk_starts[c]
                yTn = yTn_d.pop(c)
                for b in range(nbx):
                    if b == 0 and c + 2 < nchunk:
                        yT[c + 2] = load_chunk(
                            y[:], chunk_starts[c + 2], chunk_cols[c + 2]
                        )
                    if b == 1 and c + 2 < nchunk:
                        yTn_d[c + 2] = norm_chunk(yT[c + 2], chunk_cols[c + 2])
                    lhs = xTn[:, b * P : (b + 1) * P]
                    ob = obufp.tile([P, CHUNK], BF16, tag="ob")
                    for h in range(0, cols, MMCOLS):
                        hc = min(MMCOLS, cols - h)
                        ps = mpsum.tile([P, MMCOLS], F32, tag="ps")
                        for j in range(0, hc, NT):
                            nc.tensor.matmul(
                                ps[:, j : j + NT],
                                lhs,
                                yTn[:, h + j : h + j + NT],
                                start=True,
                                stop=True,
                            )
                        # Whole block drains on one engine (alternating per
                        # block) -> stores wait on a single producer.
                        if b % 2 == 0:
                            nc.vector.tensor_copy(ob[:, h : h + hc], ps[:, :hc])
                        else:
                            nc.scalar.copy(ob[:, h : h + hc], ps[:, :hc])
                    nc.sync.dma_start(
                        out=out[b * P : (b + 1) * P, col0 : col0 + cols],
                        in_=ob[:, :cols],
                    )

    nc.finalize()
    return nc


_NC_CACHE: dict[tuple[int, int], bass.Bass] = {}


def run_spmd(input1: np.ndarray, input2: np.ndarray, **kwargs):
    """Shard, run on 8 cores, gather. Returns (output, BassKernelResults)."""
    x_bf = np.asarray(input1, dtype=np.float32).astype(ml_dtypes.bfloat16)
    y_bf = np.ascontiguousarray(
        np.asarray(input2, dtype=np.float32).astype(ml_dtypes.bfloat16)
    )
    n, d = x_bf.shape
    m, d2 = y_bf.shape
    assert d == D and d2 == D and n % N_CORES == 0
    rows = n // N_CORES

    key = (rows, m)
    if key not in _NC_CACHE:
        _NC_CACHE[key] = build_nc(rows, m)
    nc = _NC_CACHE[key]

    in_maps = [
        {"x": np.ascontiguousarray(x_bf[c * rows : (c + 1) * rows]), "y": y_bf}
        for c in range(N_CORES)
    ]
    res = run_bass_kernel_spmd(nc, in_maps, core_ids=list(range(N_CORES)), **kwargs)
    out = np.concatenate(
        [res.results[c]["out"].astype(np.float32) for c in range(N_CORES)], axis=0
    )
    return out, res


def kernel(input1: np.ndarray, input2: np.ndarray) -> np.ndarray:
    return run_spmd(input1, input2)[0]
